# revision 19
# baseline (speedup 1.0000x reference)
"""Multi-head causal attention on 8 TRN2 NeuronCores.

Sharding: (batch, head-group) across 8 cores — core c handles batch c//4 and
heads [4*(c%4), 4*(c%4)+4). After attention, AllToAll exchanges per-head
attention outputs so core c computes the final output projection for rows
[512*(c%4), 512*(c%4)+512) of batch c//4. Host-side unshard is concatenation.

All projections (Q, K, V) run in fp8e4 DoubleRow off a single chunked fp8
copy of x^T (weights pre-scaled by 16 so quantization stays in fp8 normals;
QK compensation in the softmax exp scale, V compensation folded into Wo/16).
x8 is loaded in four 512-column chunks so the first projection matmuls start
as soon as ~0.75MB of input has landed instead of after the full input set.

Softmax runs without max-subtraction; the denominator comes from a ones
column appended to V. Unnormalized av rows + denominator rows travel through
FOUR AllToAlls (2 per head pair, each covering two q-chunks) so the tail
collective is small and mostly hidden. Drains are 2 DMAs per chunk ([2,65,512]
3D APs). The gpsimd queue carries ONLY the collective triggers; the receiving
gathers run on the sync queue with a sync-engine register, so the triggers
are never stuck behind register-indexed gather DMAs. On the receiver,
denominators land on 32-aligned partition pairs, one fast-approx reciprocal
inverts all of them, and four K=2 selector matmuls broadcast them across the
128 output partitions.

The attention inner loop is software-pipelined: the (c,j+1) score matmuls are
emitted before the (c,j) AV matmuls so the in-order PE never stalls waiting
for the ACT exp. A short burst of dummy matmuls at kernel start keeps the PE
busy during the initial DMA window so the first real matmuls run warm.
"""
import numpy as np
import ml_dtypes

B, S, D, H = 2, 2048, 1024, 16
DH = D // H          # 64
DIM_K = 1024
NCORES = 8
HC = 4               # heads per core
C = HC * DH          # 256 dh-columns per core
NQC = 4              # q-chunks of 512
QCH = 512
NKT = 16             # k-tiles of 128
NDC = 8              # d-chunks of 128
SCALE = float(DIM_K) ** -0.5  # 1/32
W_SCALE = 16.0       # fp8 weight pre-scale
N_WARM = 8           # dummy warm-up matmuls spanning the initial DMA window

_cache = {}


def _make_sel():
    sel = np.zeros((128, 128), np.float32)
    for i in range(4):
        sel[32 * i, 0:64] = 1.0
        sel[32 * i + 1, 64:128] = 1.0
    return sel


def _emit_body(nc, tc, pools, ins, it):
    import concourse.bass as bass
    from concourse import mybir

    f32 = mybir.dt.float32
    f32r = mybir.dt.float32r
    bf16 = mybir.dt.bfloat16
    f8 = mybir.dt.float8e4
    EXP = mybir.ActivationFunctionType.Exp
    DR = mybir.MatmulPerfMode.DoubleRow

    (persist, exps, aop, recips, osb, ps_big, ps_av, dram) = pools
    (x8_in, xt_in, wq_in, wk_in, wv_in, wo_in, tri_in, sel_in, info_in,
     out) = ins

    # ---------------- Phase A0: PE warm-up ----------------
    warm = persist.tile([64, QCH], bf16, name=f"warm_{it}", tag="warm")
    nc.vector.memset(warm[:], 1.0)
    wps = ps_big.tile([64, QCH], f32, tag="big", name=f"wps_{it}")
    for k in range(N_WARM):
        nc.tensor.matmul(wps[:], lhsT=warm[:, 0:64], rhs=warm[:],
                         start=True, stop=True)

    # ---------------- Phase A: loads ----------------
    # Weight loads on the scalar queue; x8 chunks on the sync queue in
    # consumption order; wo (needed last) on the vector queue.
    wq_sb = persist.tile([128, NDC, C], f8, name=f"wq_sb_{it}", tag="wq_sb")
    wk_sb = persist.tile([128, NDC, C], f8, name=f"wk_sb_{it}", tag="wk_sb")
    wv_sb = persist.tile([128, NDC, C], bf16, name=f"wv_sb_{it}", tag="wv_sb")
    wo_sb = persist.tile([128, NDC, DIM_K], bf16, name=f"wo_sb_{it}",
                         tag="wo_sb")
    nc.scalar.dma_start(out=wq_sb[:], in_=wq_in.ap())
    nc.scalar.dma_start(out=wk_sb[:], in_=wk_in.ap())
    nc.scalar.dma_start(out=wv_sb[:], in_=wv_in.ap())
    x8c = []
    for qc in range(NQC):
        t = persist.tile([128, NDC, QCH], f8, name=f"x8c{qc}_{it}",
                         tag=f"x8c{qc}")
        nc.sync.dma_start(out=t[:], in_=x8_in[:, qc])
        x8c.append(t)
    xtc = []
    for sc_ in range(NQC):
        t = persist.tile([128, NDC, QCH], bf16, name=f"xtc{sc_}_{it}",
                         tag=f"xtc{sc_}")
        nc.sync.dma_start(out=t[:], in_=xt_in[:, sc_])
        xtc.append(t)
    tri = persist.tile([128, 128], bf16, name=f"tri_{it}", tag="tri")
    nc.scalar.dma_start(out=tri[:], in_=tri_in.ap())
    # denominator staging tiles (memset early so the full-tile reciprocal
    # in Phase D reads benign values in the unused partition rows)
    den_sb2 = []
    for par in range(2):
        dt_ = persist.tile([128, QCH], bf16, name=f"den{par}_{it}",
                           tag=f"den{par}")
        nc.vector.memset(dt_[:], 1.0)
        den_sb2.append(dt_)
    sel = persist.tile([128, 128], f32r, name=f"sel_{it}", tag="sel")
    nc.scalar.dma_start(out=sel[:], in_=sel_in.ap())
    nc.sync.dma_start(out=wo_sb[:], in_=wo_in.ap())

    # receiver gather base register (used in Phase D)
    GATHER_ENG = "sync"
    geng = getattr(nc, GATHER_ENG)
    gbase = geng.alloc_register(f"gbase_{it}")
    geng.reg_load(gbase, info_in[0:1, 0:1])
    gbase_sv = geng.snap(gbase, donate=True, min_val=0,
                         max_val=NCORES - HC)

    # ---------------- Phase B: QKV projections ----------------
    # Q^T / K^T pair tiles chunked by q-column block: qt[p][qc] = [128, 512],
    # heads (2p, 2p+1) at partitions [0,64) / [64,128).
    qt = [[None] * NQC for _ in range(2)]
    kt = [[None] * NQC for _ in range(2)]
    for qc in range(NQC):
        for p in range(2):
            for w_sb, dst, wn in ((wq_sb, qt, "q"), (wk_sb, kt, "k")):
                ps = ps_big.tile([128, QCH], f32, tag="big",
                                 name=f"qkps{p}_{qc}_{wn}_{it}")
                for u in range(NDC // 2):
                    nc.tensor.matmul(
                        ps[:],
                        lhsT=w_sb[:, 2 * u:2 * u + 2, 128 * p:128 * (p + 1)],
                        rhs=x8c[qc][:, 2 * u:2 * u + 2, :],
                        start=(u == 0), stop=(u == NDC // 2 - 1),
                        perf_mode=DR,
                    )
                t = persist.tile([128, QCH], bf16,
                                 name=f"{wn}t{p}_{qc}_{it}",
                                 tag=f"{wn}t{p}_{qc}")
                nc.vector.tensor_copy(t[:], ps[:])
                dst[p][qc] = t
                if _cache.get("debug") and p == 0 and qc == 0:
                    dbg = _cache["dbg_tensors"]
                    nc.scalar.dma_start(
                        out=dbg["d_qt" if wn == "q" else "d_kt"].ap(),
                        in_=t[:])

    # V natural + ones column: per k-tile i, [128, 4, 65] (bf16 for
    # accuracy). Only the first 4 tiles are emitted up front; the rest are
    # interleaved into attention pair-0's steps to fill PE slack while the
    # ACT exp is the bottleneck.
    vp = [None] * NKT

    def emit_v(i):
        t = persist.tile([128, HC, DH + 1], bf16, name=f"vp{i}_{it}",
                         tag=f"vp{i}")
        nc.vector.memset(t[:, :, DH:DH + 1], 1.0)
        ps = ps_big.tile([128, C], f32, tag="big", name=f"vps{i}_{it}")
        r = 128 * (i % 4)
        for j in range(NDC):
            nc.tensor.matmul(
                ps[:],
                lhsT=xtc[i // 4][:, j, r:r + 128],
                rhs=wv_sb[:, j, :],
                start=(j == 0), stop=(j == NDC - 1),
            )
        nc.vector.tensor_copy(
            t[:, :, 0:DH], ps[:].rearrange("p (h d) -> p h d", h=HC))
        vp[i] = t
        if _cache.get("debug") and i == 0:
            nc.scalar.dma_start(
                out=_cache["dbg_tensors"]["d_vp"].ap(),
                in_=t[:].rearrange("p h d -> p (h d)"))

    for i in range(4):
        emit_v(i)
    # emission schedule for remaining V tiles inside attention pair 0:
    # step idx -> tile (tile 4(c+1)..4(c+1)+3 ready before chunk c+1 needs it)
    V_SCHED = {0: 4, 1: 5, 2: 6, 3: 7, 5: 8, 7: 9, 9: 10, 11: 11,
               14: 12, 17: 13, 20: 14, 23: 15}

    # ---------------- Phase C: attention ----------------
    # One AllToAll per head pair: blocks [8, 2, 65, 512], block = 4*bb + c
    # (destination rank). Only same-batch blocks are read by receivers.
    big_in = [dram.tile([NCORES, 2, DH + 1, QCH], bf16,
                        name=f"a2a_in{p}_{it}", tag=f"a2a_in{p}")
              for p in range(2)]
    big_out = [dram.tile([NCORES, 2, DH + 1, QCH], bf16,
                         name=f"a2a_out{p}_{it}", tag=f"a2a_out{p}")
               for p in range(2)]

    def emit_attention(p, v_sched=None):
        steps = [(c, j) for c in range(NQC) for j in range(4 * c + 4)]
        sc_views = {}
        avs_by_c = {}

        def emit_sc(idx):
            c, j = steps[idx]
            off = max(0, 128 * j - QCH * c)
            jq, jr = divmod(j, 4)
            sc = ps_big.tile([128, 2 * QCH], f32, tag="big",
                             name=f"sc{p}_{c}_{j}_{it}")
            sc3 = sc[:].rearrange("p (h n) -> p h n", h=2)
            for h2 in range(2):
                nc.tensor.matmul(
                    sc3[:, h2, off:QCH],
                    lhsT=kt[p][jq][64 * h2:64 * (h2 + 1),
                                   128 * jr:128 * (jr + 1)],
                    rhs=qt[p][c][64 * h2:64 * (h2 + 1), off:QCH],
                    start=True, stop=True,
                )
            sc_views[idx] = (sc3, off)

        def emit_exp_av(idx):
            c, j = steps[idx]
            njt = 4 * c + 4
            sc3, off = sc_views.pop(idx)
            ex = exps.tile([128, 2, QCH], bf16, tag="ex",
                           name=f"ex{p}_{c}_{j}_{it}")
            nc.scalar.activation(
                out=ex[:, :, off:QCH], in_=sc3[:, :, off:QCH],
                func=EXP, scale=SCALE / (W_SCALE * W_SCALE))
            if j // 4 == c:
                nc.vector.tensor_mul(
                    ex[:, :, off:off + 128],
                    ex[:, :, off:off + 128],
                    tri[:].unsqueeze(1).to_broadcast([128, 2, 128]),
                )
            if _cache.get("debug") and p == 0 and c == 0 and j == 0:
                nc.scalar.dma_start(
                    out=_cache["dbg_tensors"]["d_ex"].ap(),
                    in_=ex[:].rearrange("p h n -> p (h n)"))
            if j == 0:
                avs_by_c[c] = ps_av.tile([DH + 1, 2, QCH], f32, tag="av",
                                         name=f"av{p}_{c}_{it}")
            for h2 in range(2):
                nc.tensor.matmul(
                    avs_by_c[c][:, h2, off:QCH],
                    lhsT=vp[j][:, 2 * p + h2, :],
                    rhs=ex[:, h2, off:QCH],
                    start=(j == 0), stop=(j == njt - 1),
                )
            if j == njt - 1:
                emit_drain(c)

        def emit_drain(c):
            # ship UNNORMALIZED av + denominator rows; division happens on
            # the receiving core. Write both batches' candidate blocks; the
            # wrong-batch block is ignored by its receiver.
            av = avs_by_c.pop(c)
            av_sb = aop.tile([DH + 1, 2, QCH], bf16, tag="av_sb",
                             name=f"avsb{p}_{c}_{it}")
            nc.vector.tensor_copy(av_sb[:], av[:])
            if _cache.get("debug") and p == 0 and c == 0:
                nc.scalar.dma_start(
                    out=_cache["dbg_tensors"]["d_av"].ap(),
                    in_=av_sb[:].rearrange("d h n -> d (h n)"))
            for bb in range(2):
                nc.scalar.dma_start(
                    out=big_in[p][HC * bb + c]
                        .rearrange("h d n -> d h n"),
                    in_=av_sb[:])
            if c == NQC - 1:
                nc.gpsimd.collective_compute(
                    "AllToAll",
                    mybir.AluOpType.bypass,
                    replica_groups=[list(range(NCORES))],
                    ins=[big_in[p][:].opt()],
                    outs=[big_out[p][:].opt()],
                )

        emit_sc(0)
        for idx in range(len(steps)):
            if idx + 1 < len(steps):
                emit_sc(idx + 1)
            emit_exp_av(idx)
            if v_sched and idx in v_sched:
                emit_v(v_sched[idx])

    emit_attention(0, V_SCHED)
    emit_attention(1)

    # ---------------- Phase D: out projection ----------------
    # Split by head-pair parity: pair-0's half of the accumulation runs under
    # the exposed tail of AllToAll #1; pair-1's half accumulates afterwards
    # and the sum is written out, spread across DMA queues.
    o_part = []
    aoT = [None, None]
    for par in range(2):
        # gather my 4 same-batch blocks (static layout, sync queue):
        # aoT[par] = [128 = (h d), 4 sender-slots, 512]
        t = persist.tile([128, HC, QCH], bf16, name=f"aoT{par}_{it}",
                         tag=f"aoT{par}")
        for h2 in range(2):
            src = big_out[par][:][bass.ds(gbase_sv, HC),
                                  h2:h2 + 1, 0:DH, :]
            geng.dma_start(
                out=t[64 * h2:64 * (h2 + 1), :, :],
                in_=src.rearrange("b h d n -> (h d) b n"))
        aoT[par] = t
        # denominators: rows {32i, 32i+1} = (sender i, head h2)
        den_sb = den_sb2[par]
        for h2 in range(2):
            dsrc = big_out[par][:][bass.ds(gbase_sv, HC),
                                   h2:h2 + 1, DH:DH + 1, :]
            geng.dma_start(
                out=den_sb[:].rearrange("(b r) n -> b r n", r=32)
                             [:, h2:h2 + 1, :],
                in_=dsrc.rearrange("b h d n -> b (h d) n"))
        denf = recips.tile([128, QCH], f32, tag="denf", name=f"denf{par}_{it}")
        dens = recips.tile([128, QCH], f32, tag="dens", name=f"dens{par}_{it}")
        den_r = recips.tile([128, QCH], f32r, tag="denr",
                            name=f"denr{par}_{it}")
        if _cache.get("debug") and par == 0:
            nc.scalar.dma_start(
                out=_cache["dbg_tensors"]["d_den"].ap(), in_=den_sb[:])
            nc.scalar.dma_start(
                out=_cache["dbg_tensors"]["d_bin"].ap(),
                in_=big_in[0][0].rearrange("h d n -> (h d) n"))
            nc.scalar.dma_start(
                out=_cache["dbg_tensors"]["d_bout"].ap(),
                in_=big_out[0][0].rearrange("h d n -> (h d) n"))
            nc.scalar.dma_start(
                out=_cache["dbg_tensors"]["d_aoT"].ap(),
                in_=t[:].rearrange("p s n -> p (s n)"))
        nc.vector.tensor_copy(denf[:], den_sb[:])
        nc.vector.reciprocal_approx_fast(out=dens[:], in_=denf[:])
        with nc.allow_low_precision(reason="f32->f32r rounding for PE"):
            nc.vector.tensor_copy(den_r[:], dens[:])
        den_rr = den_r[:]
        for i in range(HC):
            bc = ps_big.tile([128, QCH], f32, tag="big",
                             name=f"bcps{par}_{i}_{it}")
            nc.tensor.matmul(
                bc[:],
                lhsT=sel[32 * i:32 * i + 2, :],
                rhs=den_rr[32 * i:32 * i + 2, :],
                start=True, stop=True,
                tile_position=(32 * i, 0),
            )
            nc.vector.tensor_mul(t[:, i, :], t[:, i, :], bc[:])
        if par == 0:
            for t4 in range(4):
                op_t = osb.tile([128, DIM_K], f32, tag="osb",
                                name=f"opart{t4}_{it}")
                o_part.append(op_t)
                for oc in range(2):
                    ps = ps_big.tile([128, QCH], f32, tag="big",
                                     name=f"ops0_{t4}_{oc}_{it}")
                    for k2 in range(HC):
                        nc.tensor.matmul(
                            ps[:],
                            lhsT=aoT[0][:, k2, 128 * t4:128 * (t4 + 1)],
                            rhs=wo_sb[:, 2 * k2, QCH * oc:QCH * (oc + 1)],
                            start=(k2 == 0), stop=(k2 == HC - 1),
                        )
                    nc.vector.tensor_copy(
                        op_t[:, QCH * oc:QCH * (oc + 1)], ps[:])
        else:
            add_eng = [nc.vector, nc.vector]
            oq = [nc.sync, nc.scalar]
            for t4 in range(4):
                for oc in range(2):
                    ps = ps_big.tile([128, QCH], f32, tag="big",
                                     name=f"ops1_{t4}_{oc}_{it}")
                    for k2 in range(HC):
                        nc.tensor.matmul(
                            ps[:],
                            lhsT=aoT[1][:, k2, 128 * t4:128 * (t4 + 1)],
                            rhs=wo_sb[:, 2 * k2 + 1, QCH * oc:QCH * (oc + 1)],
                            start=(k2 == 0), stop=(k2 == HC - 1),
                        )
                    add_eng[oc].tensor_add(
                        o_part[t4][:, QCH * oc:QCH * (oc + 1)],
                        o_part[t4][:, QCH * oc:QCH * (oc + 1)],
                        ps[:])
                    oq[oc].dma_start(
                        out=out[128 * t4:128 * (t4 + 1),
                                QCH * oc:QCH * (oc + 1)],
                        in_=o_part[t4][:, QCH * oc:QCH * (oc + 1)])


def _build(dup=1):
    import concourse.tile as tile
    from concourse import bacc, mybir

    f32 = mybir.dt.float32
    bf16 = mybir.dt.bfloat16
    f32r = mybir.dt.float32r
    f8 = mybir.dt.float8e4

    nc = bacc.Bacc("TRN2", target_bir_lowering=False, debug=False,
                   num_devices=NCORES)

    x8_in = nc.dram_tensor("x8", [128, NQC, NDC, QCH], f8,
                           kind="ExternalInput")
    xt_in = nc.dram_tensor("xt", [128, NQC, NDC, QCH], bf16,
                           kind="ExternalInput")
    wq_in = nc.dram_tensor("wq", [128, NDC, C], f8, kind="ExternalInput")
    wk_in = nc.dram_tensor("wk", [128, NDC, C], f8, kind="ExternalInput")
    wv_in = nc.dram_tensor("wv", [128, NDC, C], bf16, kind="ExternalInput")
    wo_in = nc.dram_tensor("wo", [128, NDC, DIM_K], bf16,
                           kind="ExternalInput")
    tri_in = nc.dram_tensor("trimask", [128, 128], bf16, kind="ExternalInput")
    sel_in = nc.dram_tensor("selmat", [128, 128], f32r, kind="ExternalInput")
    info_in = nc.dram_tensor("coreinfo", [1, 2], mybir.dt.uint32,
                             kind="ExternalInput")
    out = nc.dram_tensor("out", [QCH, DIM_K], f32, kind="ExternalOutput")
    ins = (x8_in, xt_in, wq_in, wk_in, wv_in, wo_in, tri_in, sel_in,
           info_in, out)
    if _cache.get("debug"):
        dbg = {
            "d_qt": nc.dram_tensor("d_qt", [128, QCH], bf16, kind="ExternalOutput"),
            "d_kt": nc.dram_tensor("d_kt", [128, QCH], bf16, kind="ExternalOutput"),
            "d_vp": nc.dram_tensor("d_vp", [128, HC * (DH + 1)], bf16, kind="ExternalOutput"),
            "d_ex": nc.dram_tensor("d_ex", [128, 2 * QCH], bf16, kind="ExternalOutput"),
            "d_av": nc.dram_tensor("d_av", [DH + 1, 2 * QCH], bf16, kind="ExternalOutput"),
            "d_aoT": nc.dram_tensor("d_aoT", [128, HC * QCH], bf16, kind="ExternalOutput"),
            "d_den": nc.dram_tensor("d_den", [128, QCH], bf16, kind="ExternalOutput"),
            "d_bin": nc.dram_tensor("d_bin", [2 * (DH + 1), QCH], bf16, kind="ExternalOutput"),
            "d_bout": nc.dram_tensor("d_bout", [2 * (DH + 1), QCH], bf16, kind="ExternalOutput"),
        }
        _cache["dbg_tensors"] = dbg

    with tile.TileContext(nc) as tc:
        with (
            tc.tile_pool(name="persist", bufs=1) as persist,
            tc.tile_pool(name="exps", bufs=6) as exps,
            tc.tile_pool(name="aop", bufs=4) as aop,
            tc.tile_pool(name="recips", bufs=2) as recips,
            tc.tile_pool(name="osb", bufs=4) as osb,
            tc.tile_pool(name="ps_big", bufs=2, space="PSUM") as ps_big,
            tc.tile_pool(name="ps_av", bufs=2, space="PSUM") as ps_av,
            tc.tile_pool(name="dram", bufs=1, space="DRAM") as dram,
        ):
            pools = (persist, exps, aop, recips, osb, ps_big, ps_av, dram)
            for it in range(dup):
                _emit_body(nc, tc, pools, ins, it)

    nc.compile()
    return nc


def _get_nc(dup=1):
    key = f"nc{dup}"
    if key not in _cache:
        _cache[key] = _build(dup)
    return _cache[key]


def _shuf(w):
    # [D_in, D_out] -> [128, D_in//128, D_out] partition-major
    return np.ascontiguousarray(
        w.reshape(NDC, 128, w.shape[1]).transpose(1, 0, 2))


def _make_in_maps(x, Wq, Wk, Wv, Wo):
    bf = ml_dtypes.bfloat16
    f8 = ml_dtypes.float8_e4m3fn

    # x^T packed as [128, qc, d-chunk, 512] per batch (fp8 + bf16 copies)
    x8p = []
    xtp = []
    for b in range(B):
        xt = np.asarray(x[b], np.float32).T          # [D, S]
        p = xt.reshape(NDC, 128, NQC, QCH).transpose(1, 2, 0, 3)
        x8p.append(np.clip(p, -240, 240).astype(f8))
        xtp.append(np.ascontiguousarray(p).astype(bf))

    wq_f = np.asarray(Wq, np.float32)
    wk_f = np.asarray(Wk, np.float32)
    wv_f = np.asarray(Wv, np.float32)
    wo_sh = _shuf(np.asarray(Wo, np.float32)).astype(bf)
    tri = np.triu(np.ones((128, 128), np.float32)).astype(bf)
    selm = _make_sel()

    def wcast(w):
        return np.clip(_shuf(w * W_SCALE), -240, 240).astype(f8)

    in_maps = []
    for c in range(NCORES):
        b, g = divmod(c, HC)
        cols = slice(C * g, C * (g + 1))
        info = np.array([[HC * b, 0]], dtype=np.uint32)
        in_maps.append({
            "x8": x8p[b],
            "xt": xtp[b],
            "wq": wcast(wq_f[:, cols]),
            "wk": wcast(wk_f[:, cols]),
            "wv": _shuf(wv_f[:, cols]).astype(bf),
            "wo": wo_sh,
            "trimask": tri,
            "selmat": selm,
            "coreinfo": info,
        })
    return in_maps


def kernel(x, Wq, Wk, Wv, Wo, _dup=1, _trace=False, _trace_kwargs=None):
    from concourse.bass_utils import run_bass_kernel_spmd

    in_maps = _make_in_maps(x, Wq, Wk, Wv, Wo)
    nc = _get_nc(_dup)
    res = run_bass_kernel_spmd(
        nc, in_maps, list(range(NCORES)),
        trace=_trace, **(_trace_kwargs or {}))
    _cache["last_result"] = res

    outp = np.empty((B, S, DIM_K), np.float32)
    for c in range(NCORES):
        b, g = divmod(c, HC)
        outp[b, QCH * g:QCH * (g + 1), :] = res.results[c]["out"]
    return outp


# revision 20
# speedup vs baseline: 1.0527x; 1.0527x over previous
"""Multi-head causal attention on 8 TRN2 NeuronCores.

Sharding: (batch, head-group) across 8 cores — core c handles batch c//4 and
heads [4*(c%4), 4*(c%4)+4). After attention, AllToAll exchanges per-head
attention outputs so core c computes the final output projection for rows
[512*(c%4), 512*(c%4)+512) of batch c//4. Host-side unshard is concatenation.

All projections (Q, K, V) run in fp8e4 DoubleRow off a single chunked fp8
copy of x^T (weights pre-scaled by 16 so quantization stays in fp8 normals;
QK compensation in the softmax exp scale, V compensation folded into Wo/16).
x8 is loaded in four 512-column chunks so the first projection matmuls start
as soon as ~0.75MB of input has landed instead of after the full input set.

Softmax runs without max-subtraction; the denominator comes from a ones
column appended to V. Unnormalized av rows + denominator rows travel through
FOUR AllToAlls (2 per head pair, each covering two q-chunks) so the tail
collective is small and mostly hidden. Drains are 2 DMAs per chunk ([2,65,512]
3D APs). The gpsimd queue carries ONLY the collective triggers; the receiving
gathers run on the sync queue with a sync-engine register, so the triggers
are never stuck behind register-indexed gather DMAs. On the receiver,
denominators land on 32-aligned partition pairs, one fast-approx reciprocal
inverts all of them, and four K=2 selector matmuls broadcast them across the
128 output partitions.

The attention inner loop is software-pipelined: the (c,j+1) score matmuls are
emitted before the (c,j) AV matmuls so the in-order PE never stalls waiting
for the ACT exp. A short burst of dummy matmuls at kernel start keeps the PE
busy during the initial DMA window so the first real matmuls run warm.
"""
import numpy as np
import ml_dtypes

B, S, D, H = 2, 2048, 1024, 16
DH = D // H          # 64
DIM_K = 1024
NCORES = 8
HC = 4               # heads per core
C = HC * DH          # 256 dh-columns per core
NQC = 4              # q-chunks of 512
QCH = 512
NKT = 16             # k-tiles of 128
NDC = 8              # d-chunks of 128
SCALE = float(DIM_K) ** -0.5  # 1/32
W_SCALE = 16.0       # fp8 weight pre-scale
N_WARM = 4           # dummy warm-up matmuls spanning the initial DMA window

_cache = {}


def _make_sel():
    sel = np.zeros((128, 128), np.float32)
    for i in range(4):
        sel[32 * i, 0:64] = 1.0
        sel[32 * i + 1, 64:128] = 1.0
    return sel


def _emit_body(nc, tc, pools, ins, it):
    import concourse.bass as bass
    from concourse import mybir

    f32 = mybir.dt.float32
    f32r = mybir.dt.float32r
    bf16 = mybir.dt.bfloat16
    f8 = mybir.dt.float8e4
    EXP = mybir.ActivationFunctionType.Exp
    DR = mybir.MatmulPerfMode.DoubleRow

    (persist, exps, aop, recips, osb, ps_big, ps_av, dram) = pools
    (x8_in, xt_in, wq_in, wk_in, wv_in, wo_in, tri_in, sel_in, info_in,
     out) = ins

    # ---------------- Phase A0: PE warm-up ----------------
    warm = persist.tile([64, QCH], bf16, name=f"warm_{it}", tag="warm")
    nc.vector.memset(warm[:], 1.0)
    wps = ps_big.tile([64, QCH], f32, tag="big", name=f"wps_{it}")
    for k in range(N_WARM):
        nc.tensor.matmul(wps[:], lhsT=warm[:, 0:64], rhs=warm[:],
                         start=True, stop=True)

    # ---------------- Phase A: loads ----------------
    # Weight loads on the scalar queue; x8 chunks on the sync queue in
    # consumption order; wo (needed last) on the vector queue.
    wq_sb = persist.tile([128, NDC, C], f8, name=f"wq_sb_{it}", tag="wq_sb")
    wk_sb = persist.tile([128, NDC, C], f8, name=f"wk_sb_{it}", tag="wk_sb")
    wv_sb = persist.tile([128, NDC, C], bf16, name=f"wv_sb_{it}", tag="wv_sb")
    wo_sb = persist.tile([128, NDC, DIM_K], bf16, name=f"wo_sb_{it}",
                         tag="wo_sb")
    nc.scalar.dma_start(out=wq_sb[:], in_=wq_in.ap())
    nc.scalar.dma_start(out=wk_sb[:], in_=wk_in.ap())
    nc.scalar.dma_start(out=wv_sb[:], in_=wv_in.ap())
    x8c = []
    xtc = []
    for qc in range(NQC):
        t8 = persist.tile([128, NDC, QCH], f8, name=f"x8c{qc}_{it}",
                          tag=f"x8c{qc}")
        nc.sync.dma_start(out=t8[:], in_=x8_in[:, qc])
        x8c.append(t8)
        tb = persist.tile([128, NDC, QCH], bf16, name=f"xtc{qc}_{it}",
                          tag=f"xtc{qc}")
        nc.sync.dma_start(out=tb[:], in_=xt_in[:, qc])
        xtc.append(tb)
    tri = persist.tile([128, 128], bf16, name=f"tri_{it}", tag="tri")
    nc.scalar.dma_start(out=tri[:], in_=tri_in.ap())
    # denominator staging tiles (memset early so the full-tile reciprocal
    # in Phase D reads benign values in the unused partition rows)
    den_sb2 = []
    for par in range(2):
        dt_ = persist.tile([128, QCH], bf16, name=f"den{par}_{it}",
                           tag=f"den{par}")
        nc.vector.memset(dt_[:], 1.0)
        den_sb2.append(dt_)
    sel = persist.tile([128, 128], f32r, name=f"sel_{it}", tag="sel")
    nc.scalar.dma_start(out=sel[:], in_=sel_in.ap())
    nc.sync.dma_start(out=wo_sb[:], in_=wo_in.ap())

    # receiver gather base register (used in Phase D)
    GATHER_ENG = "sync"
    geng = getattr(nc, GATHER_ENG)
    gbase = geng.alloc_register(f"gbase_{it}")
    geng.reg_load(gbase, info_in[0:1, 0:1])
    gbase_sv = geng.snap(gbase, donate=True, min_val=0,
                         max_val=NCORES - HC)

    # ---------------- Phase B: QKV projections ----------------
    # Q^T / K^T pair tiles chunked by q-column block: qt[p][qc] = [128, 512],
    # heads (2p, 2p+1) at partitions [0,64) / [64,128).
    qt = [[None] * NQC for _ in range(2)]
    kt = [[None] * NQC for _ in range(2)]

    def emit_qk(qc):
        for p in range(2):
            for w_sb, dst, wn in ((wq_sb, qt, "q"), (wk_sb, kt, "k")):
                ps = ps_big.tile([128, QCH], f32, tag="big",
                                 name=f"qkps{p}_{qc}_{wn}_{it}")
                for u in range(NDC // 2):
                    nc.tensor.matmul(
                        ps[:],
                        lhsT=w_sb[:, 2 * u:2 * u + 2, 128 * p:128 * (p + 1)],
                        rhs=x8c[qc][:, 2 * u:2 * u + 2, :],
                        start=(u == 0), stop=(u == NDC // 2 - 1),
                        perf_mode=DR,
                    )
                t = persist.tile([128, QCH], bf16,
                                 name=f"{wn}t{p}_{qc}_{it}",
                                 tag=f"{wn}t{p}_{qc}")
                nc.vector.tensor_copy(t[:], ps[:])
                dst[p][qc] = t
                if _cache.get("debug") and p == 0 and qc == 0:
                    dbg = _cache["dbg_tensors"]
                    nc.scalar.dma_start(
                        out=dbg["d_qt" if wn == "q" else "d_kt"].ap(),
                        in_=t[:])

    # V natural + ones column: per k-tile i, [128, 4, 65] (bf16 for
    # accuracy). Only the first 4 tiles are emitted up front; the rest are
    # interleaved into attention pair-0's steps to fill PE slack while the
    # ACT exp is the bottleneck.
    vp = [None] * NKT

    def emit_v(i):
        t = persist.tile([128, HC, DH + 1], bf16, name=f"vp{i}_{it}",
                         tag=f"vp{i}")
        nc.vector.memset(t[:, :, DH:DH + 1], 1.0)
        ps = ps_big.tile([128, C], f32, tag="big", name=f"vps{i}_{it}")
        r = 128 * (i % 4)
        for j in range(NDC):
            nc.tensor.matmul(
                ps[:],
                lhsT=xtc[i // 4][:, j, r:r + 128],
                rhs=wv_sb[:, j, :],
                start=(j == 0), stop=(j == NDC - 1),
            )
        nc.vector.tensor_copy(
            t[:, :, 0:DH], ps[:].rearrange("p (h d) -> p h d", h=HC))
        vp[i] = t
        if _cache.get("debug") and i == 0:
            nc.scalar.dma_start(
                out=_cache["dbg_tensors"]["d_vp"].ap(),
                in_=t[:].rearrange("p h d -> p (h d)"))

    # dense per-chunk QKV: QK for chunk qc, then V k-tiles 4qc..4qc+3 —
    # keeps the PE busy back-to-back from the first chunk so the HAM clock
    # gate opens early and stays open into attention.
    for qc in range(NQC):
        emit_qk(qc)
        for i in range(4 * qc, 4 * qc + 4):
            emit_v(i)

    # ---------------- Phase C: attention ----------------
    # One AllToAll per head pair: blocks [8, 2, 65, 512], block = 4*bb + c
    # (destination rank). Only same-batch blocks are read by receivers.
    big_in = [dram.tile([NCORES, 2, DH + 1, QCH], bf16,
                        name=f"a2a_in{p}_{it}", tag=f"a2a_in{p}")
              for p in range(2)]
    big_out = [dram.tile([NCORES, 2, DH + 1, QCH], bf16,
                         name=f"a2a_out{p}_{it}", tag=f"a2a_out{p}")
               for p in range(2)]

    def emit_attention(p, v_sched=None):
        steps = [(c, j) for c in range(NQC) for j in range(4 * c + 4)]
        sc_views = {}
        avs_by_c = {}

        def emit_sc(idx):
            c, j = steps[idx]
            off = max(0, 128 * j - QCH * c)
            jq, jr = divmod(j, 4)
            sc = ps_big.tile([128, 2 * QCH], f32, tag="big",
                             name=f"sc{p}_{c}_{j}_{it}")
            sc3 = sc[:].rearrange("p (h n) -> p h n", h=2)
            for h2 in range(2):
                nc.tensor.matmul(
                    sc3[:, h2, off:QCH],
                    lhsT=kt[p][jq][64 * h2:64 * (h2 + 1),
                                   128 * jr:128 * (jr + 1)],
                    rhs=qt[p][c][64 * h2:64 * (h2 + 1), off:QCH],
                    start=True, stop=True,
                )
            sc_views[idx] = (sc3, off)

        def emit_exp_av(idx):
            c, j = steps[idx]
            njt = 4 * c + 4
            sc3, off = sc_views.pop(idx)
            ex = exps.tile([128, 2, QCH], bf16, tag="ex",
                           name=f"ex{p}_{c}_{j}_{it}")
            nc.scalar.activation(
                out=ex[:, :, off:QCH], in_=sc3[:, :, off:QCH],
                func=EXP, scale=SCALE / (W_SCALE * W_SCALE))
            if j // 4 == c:
                nc.vector.tensor_mul(
                    ex[:, :, off:off + 128],
                    ex[:, :, off:off + 128],
                    tri[:].unsqueeze(1).to_broadcast([128, 2, 128]),
                )
            if _cache.get("debug") and p == 0 and c == 0 and j == 0:
                nc.scalar.dma_start(
                    out=_cache["dbg_tensors"]["d_ex"].ap(),
                    in_=ex[:].rearrange("p h n -> p (h n)"))
            if j == 0:
                avs_by_c[c] = ps_av.tile([DH + 1, 2, QCH], f32, tag="av",
                                         name=f"av{p}_{c}_{it}")
            for h2 in range(2):
                nc.tensor.matmul(
                    avs_by_c[c][:, h2, off:QCH],
                    lhsT=vp[j][:, 2 * p + h2, :],
                    rhs=ex[:, h2, off:QCH],
                    start=(j == 0), stop=(j == njt - 1),
                )
            if j == njt - 1:
                emit_drain(c)

        def emit_drain(c):
            # ship UNNORMALIZED av + denominator rows; division happens on
            # the receiving core. Write both batches' candidate blocks; the
            # wrong-batch block is ignored by its receiver.
            av = avs_by_c.pop(c)
            av_sb = aop.tile([DH + 1, 2, QCH], bf16, tag="av_sb",
                             name=f"avsb{p}_{c}_{it}")
            nc.vector.tensor_copy(av_sb[:], av[:])
            if _cache.get("debug") and p == 0 and c == 0:
                nc.scalar.dma_start(
                    out=_cache["dbg_tensors"]["d_av"].ap(),
                    in_=av_sb[:].rearrange("d h n -> d (h n)"))
            for bb in range(2):
                nc.scalar.dma_start(
                    out=big_in[p][HC * bb + c]
                        .rearrange("h d n -> d h n"),
                    in_=av_sb[:])
            if c == NQC - 1:
                nc.gpsimd.collective_compute(
                    "AllToAll",
                    mybir.AluOpType.bypass,
                    replica_groups=[list(range(NCORES))],
                    ins=[big_in[p][:].opt()],
                    outs=[big_out[p][:].opt()],
                )

        emit_sc(0)
        for idx in range(len(steps)):
            if idx + 1 < len(steps):
                emit_sc(idx + 1)
            emit_exp_av(idx)
            if v_sched and idx in v_sched:
                emit_v(v_sched[idx])

    emit_attention(0)
    emit_attention(1)

    # ---------------- Phase D: out projection ----------------
    # Split by head-pair parity: pair-0's half of the accumulation runs under
    # the exposed tail of AllToAll #1; pair-1's half accumulates afterwards
    # and the sum is written out, spread across DMA queues.
    o_part = []
    aoT = [None, None]
    for par in range(2):
        # gather my 4 same-batch blocks (static layout, sync queue):
        # aoT[par] = [128 = (h d), 4 sender-slots, 512]
        t = persist.tile([128, HC, QCH], bf16, name=f"aoT{par}_{it}",
                         tag=f"aoT{par}")
        for h2 in range(2):
            src = big_out[par][:][bass.ds(gbase_sv, HC),
                                  h2:h2 + 1, 0:DH, :]
            geng.dma_start(
                out=t[64 * h2:64 * (h2 + 1), :, :],
                in_=src.rearrange("b h d n -> (h d) b n"))
        aoT[par] = t
        # denominators: rows {32i, 32i+1} = (sender i, head h2)
        den_sb = den_sb2[par]
        for h2 in range(2):
            dsrc = big_out[par][:][bass.ds(gbase_sv, HC),
                                   h2:h2 + 1, DH:DH + 1, :]
            geng.dma_start(
                out=den_sb[:].rearrange("(b r) n -> b r n", r=32)
                             [:, h2:h2 + 1, :],
                in_=dsrc.rearrange("b h d n -> b (h d) n"))
        denf = recips.tile([128, QCH], f32, tag="denf", name=f"denf{par}_{it}")
        dens = recips.tile([128, QCH], f32, tag="dens", name=f"dens{par}_{it}")
        den_r = recips.tile([128, QCH], f32r, tag="denr",
                            name=f"denr{par}_{it}")
        if _cache.get("debug") and par == 0:
            nc.scalar.dma_start(
                out=_cache["dbg_tensors"]["d_den"].ap(), in_=den_sb[:])
            nc.scalar.dma_start(
                out=_cache["dbg_tensors"]["d_bin"].ap(),
                in_=big_in[0][0].rearrange("h d n -> (h d) n"))
            nc.scalar.dma_start(
                out=_cache["dbg_tensors"]["d_bout"].ap(),
                in_=big_out[0][0].rearrange("h d n -> (h d) n"))
            nc.scalar.dma_start(
                out=_cache["dbg_tensors"]["d_aoT"].ap(),
                in_=t[:].rearrange("p s n -> p (s n)"))
        nc.vector.tensor_copy(denf[:], den_sb[:])
        nc.vector.reciprocal_approx_fast(out=dens[:], in_=denf[:])
        with nc.allow_low_precision(reason="f32->f32r rounding for PE"):
            nc.vector.tensor_copy(den_r[:], dens[:])
        den_rr = den_r[:]
        for i in range(HC):
            bc = ps_big.tile([128, QCH], f32, tag="big",
                             name=f"bcps{par}_{i}_{it}")
            nc.tensor.matmul(
                bc[:],
                lhsT=sel[32 * i:32 * i + 2, :],
                rhs=den_rr[32 * i:32 * i + 2, :],
                start=True, stop=True,
                tile_position=(32 * i, 0),
            )
            nc.vector.tensor_mul(t[:, i, :], t[:, i, :], bc[:])
        if par == 0:
            for t4 in range(4):
                op_t = osb.tile([128, DIM_K], f32, tag="osb",
                                name=f"opart{t4}_{it}")
                o_part.append(op_t)
                for oc in range(2):
                    ps = ps_big.tile([128, QCH], f32, tag="big",
                                     name=f"ops0_{t4}_{oc}_{it}")
                    for k2 in range(HC):
                        nc.tensor.matmul(
                            ps[:],
                            lhsT=aoT[0][:, k2, 128 * t4:128 * (t4 + 1)],
                            rhs=wo_sb[:, 2 * k2, QCH * oc:QCH * (oc + 1)],
                            start=(k2 == 0), stop=(k2 == HC - 1),
                        )
                    nc.vector.tensor_copy(
                        op_t[:, QCH * oc:QCH * (oc + 1)], ps[:])
        else:
            add_eng = [nc.vector, nc.vector]
            oq = [nc.sync, nc.scalar]
            for t4 in range(4):
                for oc in range(2):
                    ps = ps_big.tile([128, QCH], f32, tag="big",
                                     name=f"ops1_{t4}_{oc}_{it}")
                    for k2 in range(HC):
                        nc.tensor.matmul(
                            ps[:],
                            lhsT=aoT[1][:, k2, 128 * t4:128 * (t4 + 1)],
                            rhs=wo_sb[:, 2 * k2 + 1, QCH * oc:QCH * (oc + 1)],
                            start=(k2 == 0), stop=(k2 == HC - 1),
                        )
                    add_eng[oc].tensor_add(
                        o_part[t4][:, QCH * oc:QCH * (oc + 1)],
                        o_part[t4][:, QCH * oc:QCH * (oc + 1)],
                        ps[:])
                    oq[oc].dma_start(
                        out=out[128 * t4:128 * (t4 + 1),
                                QCH * oc:QCH * (oc + 1)],
                        in_=o_part[t4][:, QCH * oc:QCH * (oc + 1)])


def _build(dup=1):
    import concourse.tile as tile
    from concourse import bacc, mybir

    f32 = mybir.dt.float32
    bf16 = mybir.dt.bfloat16
    f32r = mybir.dt.float32r
    f8 = mybir.dt.float8e4

    nc = bacc.Bacc("TRN2", target_bir_lowering=False, debug=False,
                   num_devices=NCORES)

    x8_in = nc.dram_tensor("x8", [128, NQC, NDC, QCH], f8,
                           kind="ExternalInput")
    xt_in = nc.dram_tensor("xt", [128, NQC, NDC, QCH], bf16,
                           kind="ExternalInput")
    wq_in = nc.dram_tensor("wq", [128, NDC, C], f8, kind="ExternalInput")
    wk_in = nc.dram_tensor("wk", [128, NDC, C], f8, kind="ExternalInput")
    wv_in = nc.dram_tensor("wv", [128, NDC, C], bf16, kind="ExternalInput")
    wo_in = nc.dram_tensor("wo", [128, NDC, DIM_K], bf16,
                           kind="ExternalInput")
    tri_in = nc.dram_tensor("trimask", [128, 128], bf16, kind="ExternalInput")
    sel_in = nc.dram_tensor("selmat", [128, 128], f32r, kind="ExternalInput")
    info_in = nc.dram_tensor("coreinfo", [1, 2], mybir.dt.uint32,
                             kind="ExternalInput")
    out = nc.dram_tensor("out", [QCH, DIM_K], f32, kind="ExternalOutput")
    ins = (x8_in, xt_in, wq_in, wk_in, wv_in, wo_in, tri_in, sel_in,
           info_in, out)
    if _cache.get("debug"):
        dbg = {
            "d_qt": nc.dram_tensor("d_qt", [128, QCH], bf16, kind="ExternalOutput"),
            "d_kt": nc.dram_tensor("d_kt", [128, QCH], bf16, kind="ExternalOutput"),
            "d_vp": nc.dram_tensor("d_vp", [128, HC * (DH + 1)], bf16, kind="ExternalOutput"),
            "d_ex": nc.dram_tensor("d_ex", [128, 2 * QCH], bf16, kind="ExternalOutput"),
            "d_av": nc.dram_tensor("d_av", [DH + 1, 2 * QCH], bf16, kind="ExternalOutput"),
            "d_aoT": nc.dram_tensor("d_aoT", [128, HC * QCH], bf16, kind="ExternalOutput"),
            "d_den": nc.dram_tensor("d_den", [128, QCH], bf16, kind="ExternalOutput"),
            "d_bin": nc.dram_tensor("d_bin", [2 * (DH + 1), QCH], bf16, kind="ExternalOutput"),
            "d_bout": nc.dram_tensor("d_bout", [2 * (DH + 1), QCH], bf16, kind="ExternalOutput"),
        }
        _cache["dbg_tensors"] = dbg

    with tile.TileContext(nc) as tc:
        with (
            tc.tile_pool(name="persist", bufs=1) as persist,
            tc.tile_pool(name="exps", bufs=6) as exps,
            tc.tile_pool(name="aop", bufs=4) as aop,
            tc.tile_pool(name="recips", bufs=2) as recips,
            tc.tile_pool(name="osb", bufs=4) as osb,
            tc.tile_pool(name="ps_big", bufs=2, space="PSUM") as ps_big,
            tc.tile_pool(name="ps_av", bufs=2, space="PSUM") as ps_av,
            tc.tile_pool(name="dram", bufs=1, space="DRAM") as dram,
        ):
            pools = (persist, exps, aop, recips, osb, ps_big, ps_av, dram)
            for it in range(dup):
                _emit_body(nc, tc, pools, ins, it)

    nc.compile()
    return nc


def _get_nc(dup=1):
    key = f"nc{dup}"
    if key not in _cache:
        _cache[key] = _build(dup)
    return _cache[key]


def _shuf(w):
    # [D_in, D_out] -> [128, D_in//128, D_out] partition-major
    return np.ascontiguousarray(
        w.reshape(NDC, 128, w.shape[1]).transpose(1, 0, 2))


def _make_in_maps(x, Wq, Wk, Wv, Wo):
    bf = ml_dtypes.bfloat16
    f8 = ml_dtypes.float8_e4m3fn

    # x^T packed as [128, qc, d-chunk, 512] per batch (fp8 + bf16 copies)
    x8p = []
    xtp = []
    for b in range(B):
        xt = np.asarray(x[b], np.float32).T          # [D, S]
        p = xt.reshape(NDC, 128, NQC, QCH).transpose(1, 2, 0, 3)
        x8p.append(np.clip(p, -240, 240).astype(f8))
        xtp.append(np.ascontiguousarray(p).astype(bf))

    wq_f = np.asarray(Wq, np.float32)
    wk_f = np.asarray(Wk, np.float32)
    wv_f = np.asarray(Wv, np.float32)
    wo_sh = _shuf(np.asarray(Wo, np.float32)).astype(bf)
    tri = np.triu(np.ones((128, 128), np.float32)).astype(bf)
    selm = _make_sel()

    def wcast(w):
        return np.clip(_shuf(w * W_SCALE), -240, 240).astype(f8)

    in_maps = []
    for c in range(NCORES):
        b, g = divmod(c, HC)
        cols = slice(C * g, C * (g + 1))
        info = np.array([[HC * b, 0]], dtype=np.uint32)
        in_maps.append({
            "x8": x8p[b],
            "xt": xtp[b],
            "wq": wcast(wq_f[:, cols]),
            "wk": wcast(wk_f[:, cols]),
            "wv": _shuf(wv_f[:, cols]).astype(bf),
            "wo": wo_sh,
            "trimask": tri,
            "selmat": selm,
            "coreinfo": info,
        })
    return in_maps


def kernel(x, Wq, Wk, Wv, Wo, _dup=1, _trace=False, _trace_kwargs=None):
    from concourse.bass_utils import run_bass_kernel_spmd

    in_maps = _make_in_maps(x, Wq, Wk, Wv, Wo)
    nc = _get_nc(_dup)
    res = run_bass_kernel_spmd(
        nc, in_maps, list(range(NCORES)),
        trace=_trace, **(_trace_kwargs or {}))
    _cache["last_result"] = res

    outp = np.empty((B, S, DIM_K), np.float32)
    for c in range(NCORES):
        b, g = divmod(c, HC)
        outp[b, QCH * g:QCH * (g + 1), :] = res.results[c]["out"]
    return outp
